# revision 45
# baseline (speedup 1.0000x reference)
"""InfoNCE lower-bound kernel for 8 Trainium2 NeuronCores.

Math (reference):
  hx = x @ W1x.T ; hy = y @ W1y.T            [N, H]
  z_ij = relu(hx[j] + hy[i] + b1) . w2       (logit WITHOUT b2)
  T1[i,j] = softplus(z_ij + b2)
  T0[i]   = T1[i,i]
  lse[i]  = log(sum_j exp(T1[i,j]))
  out     = mean(T0) - (mean(lse) - log N)

Key identity used on-device:  exp(softplus(v)) = 1 + e^v, so
  lse[i] = log(N + sum_j exp(z_ij + b2))
which avoids materializing softplus over the [N, N] grid.

Sharding: data-parallel over i (rows of the pair grid). Each of the 8
cores gets 64 rows (its slice of y), x and the MLP params replicated.
Per-core partial sums of T0 and lse are combined on the host.

The big [N/8, N, H] relu+matvec grid runs in bf16: the PE matvec
streams bf16 (1 cycle/row vs 4 for fp32) and the DVE relu ops hit the
2x 16-bit mode. PSUM accumulation stays fp32.

Per 4-row PSUM bank group: 8 relu+matvec ops cover h-tiles {128,128};
the 44-wide t2 tile packs two rows per op at partitions {0:44, 64:108}
(both ranges produced directly by matmuls against host-duplicated w1
slices — no staging DMAs), and a [128,33] lhsT whose cols 0/32 carry
w2_t2 on the matching ranges lands both dots on the rows' PSUM slots.
A per-group Exp with accum_out reduces each bank straight to row sums
— z never round-trips through SBUF — and the host finishes with
ln(N + sexp) / log1p(ed) on 8x(64+64) scalars. Engine budget per
group: DVE 8 relus (~2.1us), Act 2 relus + exp (~2.1us), PE 10
matvecs (~2.1us), all three ~saturated. The first 3 groups interleave
with the preamble; PE p-state warmup runs while inputs stream in.
Measured ~60-65us on 8 cores, rel err ~7e-4.
"""

import math

import numpy as np

N = 512
XD = 768
YD = 768
H = 300
NCORES = 8
ISH = N // NCORES  # 64 rows per core
KD = XD // 128     # 6 contraction tiles of 128
HT = 3             # h tiles: 128, 128, 44
HSZ = [128, 128, H - 256]

_CACHE = {}
TRACE = False
LAST_RESULTS = None


def _build_module():
    import concourse.bacc as bacc
    import concourse.mybir as mybir
    from concourse.tile import TileContext

    f32 = mybir.dt.float32
    bf16 = mybir.dt.bfloat16
    AF = mybir.ActivationFunctionType
    ALU = mybir.AluOpType
    AX = mybir.AxisListType

    nc = bacc.Bacc("TRN2", target_bir_lowering=False, debug=False)

    # Per-core inputs (SPMD: same shapes, different data for yT/xTd).
    xT = nc.dram_tensor("xT", [XD, N], bf16, kind="ExternalInput")       # x^T
    w1xT = nc.dram_tensor("w1xT", [XD, H], bf16, kind="ExternalInput")   # W1x^T
    w1yT = nc.dram_tensor("w1yT", [YD, H], bf16, kind="ExternalInput")   # W1y^T
    yx = nc.dram_tensor("yx", [YD, 2 * ISH], bf16, kind="ExternalInput")  # [y-slice^T | x-slice^T]
    b12 = nc.dram_tensor("b12", [128, HT + 2], f32, kind="ExternalInput")  # b1 packed | b2 | b1q
    w1q = nc.dram_tensor("w1q", [XD, 216], bf16, kind="ExternalInput")   # dup t2 weights [x|y]
    w2a = nc.dram_tensor("w2a", [128, HT + 64], bf16, kind="ExternalInput")  # w2 packed | t2-pair lhsT
    out_ed = nc.dram_tensor("out_ed", [1, ISH], f32, kind="ExternalOutput")      # exp(z_ii+b2)
    out_sx = nc.dram_tensor("out_sx", [4, ISH // 4], f32, kind="ExternalOutput")  # row exp sums

    with TileContext(nc) as tc:
        cpool = tc.alloc_tile_pool(name="consts", bufs=1)
        rpool = tc.alloc_tile_pool(name="work", bufs=24)
        tpool = tc.alloc_tile_pool(name="tail", bufs=1)
        pp_warm = tc.alloc_tile_pool(name="pp_warm", bufs=1, space="PSUM")
        pp_pre = tc.alloc_tile_pool(name="pp_pre", bufs=2, space="PSUM")
        pp_z = tc.alloc_tile_pool(name="pp_z", bufs=3, space="PSUM")
        pp_misc = tc.alloc_tile_pool(name="pp_misc", bufs=1, space="PSUM")

        # ---- load constants / inputs into SBUF ----
        # One fused DMA per DRAM tensor: each dma_start costs ~580ns of issue
        # on the SP queue, so 33 small loads would serialize ~19us.
        xt_sb = cpool.tile([128, KD * N], bf16, tag="xt")
        w1x_sb = cpool.tile([128, KD * H], bf16, tag="w1x")
        w1y_sb = cpool.tile([128, KD * H], bf16, tag="w1y")
        yx_sb = cpool.tile([128, KD * 2 * ISH], bf16, tag="yx")
        b12_sb = cpool.tile([128, HT + 2], f32, tag="b12")
        w1q_sb = cpool.tile([128, KD * 216], bf16, tag="w1q")
        w2a_sb = cpool.tile([128, HT + 64], bf16, tag="w2a")

        # Issue order tracks preamble consumption: xt+w1x[t0] unblock the
        # first hx matmuls; per-tile w1x/w1y slices arrive just in time for
        # each later tile. Queues drain in issue order, so one big DMA per
        # tensor would stall tile 0 on bytes only tile 2 needs.
        nc.sync.dma_start(
            xt_sb[:].rearrange("p (k c) -> p k c", k=KD),
            xT[:, :].rearrange("(k p) c -> p k c", p=128),
        )

        def load_w1_tile(dst_sb, src_dram, t):
            hs = HSZ[t]
            nc.sync.dma_start(
                dst_sb[:].rearrange("p (k c) -> p k c", k=KD)[:, :, 128 * t:128 * t + hs],
                src_dram[:, 128 * t:128 * t + hs].rearrange("(k p) c -> p k c", p=128),
            )

        load_w1_tile(w1x_sb, w1xT, 0)
        nc.sync.dma_start(b12_sb[:], b12[:])
        nc.sync.dma_start(w2a_sb[:], w2a[:])
        load_w1_tile(w1y_sb, w1yT, 0)
        nc.sync.dma_start(
            yx_sb[:].rearrange("p (k c) -> p k c", k=KD),
            yx[:, :].rearrange("(k p) c -> p k c", p=128),
        )
        load_w1_tile(w1x_sb, w1xT, 1)
        load_w1_tile(w1y_sb, w1yT, 1)
        nc.sync.dma_start(
            w1q_sb[:].rearrange("p (k c) -> p k c", k=KD),
            w1q[:, :].rearrange("(k p) c -> p k c", p=128),
        )
        load_w1_tile(w1x_sb, w1xT, 2)

        # ---- PE p-state warmup: dummy matmuls while input DMAs land ----
        # gpsimd memset has no upstream deps, so the PE starts within ~1us
        # of NEFF start and stays continuously busy (p-state ramps to 2.4GHz)
        # until the real preamble matmuls have data.
        warm_sb = cpool.tile([128, N], bf16, tag="warm")
        nc.gpsimd.memset(warm_sb[:], 0.0)
        wps = pp_warm.tile([128, N], f32, tag="wps")
        for _ in range(17):
            nc.tensor.matmul(
                wps[0:1, :], lhsT=warm_sb[:, 0:1], rhs=warm_sb[:],
                start=True, stop=True,
            )

        # ---- precompute hxT(+b1), hyT, hxdT on device ----
        hxb_sb = cpool.tile([128, 2 * N], bf16, tag="hxb")    # relu-arg x part (+b1)
        hy_sb = cpool.tile([128, HT * ISH], f32, tag="hy")    # y part (scalar operands)
        hxd_sb = cpool.tile([128, HT * ISH], f32, tag="hxd")  # diag x part (+b1)
        nc.vector.memset(hy_sb[:, 2 * ISH:3 * ISH], 0.0)
        nc.vector.memset(hxd_sb[:, 2 * ISH:3 * ISH], 0.0)

        NG = ISH // 4
        STAG = 3  # groups whose matvecs interleave with the preamble
        T2 = HSZ[2]
        sexp_all = cpool.tile([128, NG], f32, tag="sexp_all")
        ejunk = cpool.tile([128, N], bf16, tag="ejunk")
        hxb2 = cpool.tile([128, N], bf16, tag="hxb2")
        hy2 = cpool.tile([128, ISH // 2], f32, tag="hy2")
        zps = {}

        def emit_row_t(g, k4, t):
            i = 4 * g + k4
            r = rpool.tile([128, N], bf16, name="r", tag="r")
            col = hy_sb[:, t * ISH + i: t * ISH + i + 1]
            src = hxb_sb[:, t * N:(t + 1) * N]
            if t == 1 and k4 < 2:
                nc.scalar.activation(r[:], src, AF.Relu, bias=col)
            else:
                nc.vector.tensor_scalar(r[:], src, col, 0.0, ALU.add, ALU.max)
            nc.tensor.matmul(
                zps[g][32 * k4:32 * k4 + 1, :],
                lhsT=w2a_sb[:, t:t + 1], rhs=r[:],
                start=(t == 0), stop=False,
                tile_position=(0, 32 * k4),
                skip_group_check=True,
            )

        def emit_pair(g, a):
            pp = 2 * g + a
            r2 = rpool.tile([128, N], bf16, name="r2", tag="r2")
            nc.vector.tensor_scalar(
                r2[:], hxb2[:], hy2[:, pp:pp + 1], 0.0, ALU.add, ALU.max
            )
            nc.tensor.matmul(
                zps[g][64 * a:64 * a + 33, :],
                lhsT=w2a_sb[:, HT:HT + 33], rhs=r2[:],
                start=False, stop=True,
                tile_position=(0, 64 * a),
                skip_group_check=True,
            )

        def emit_exp(g):
            nc.scalar.activation(
                ejunk[:], zps[g][:], AF.Exp, bias=b12_sb[:, HT:HT + 1],
                accum_out=sexp_all[:, g:g + 1],
            )

        # ---- preamble interleaved with the first STAG groups ----
        # hx_t/hy_t clear tile t; the staged groups' tile-t matvecs then keep
        # the PE busy while the next tile's weights are still in flight.
        for g in range(STAG):
            zps[g] = pp_z.tile([128, N], f32, name="zp", tag="zp")
        def emit_hx(t):
            hs = HSZ[t]
            ps = pp_pre.tile([128, N], f32, name="ps", tag="pre")
            for k in range(KD):
                nc.tensor.matmul(
                    ps[0:hs, :],
                    lhsT=w1x_sb[:, k * H + 128 * t: k * H + 128 * t + hs],
                    rhs=xt_sb[:, k * N:(k + 1) * N],
                    start=(k == 0), stop=(k == KD - 1),
                )
            nc.scalar.activation(
                hxb_sb[0:hs, t * N:(t + 1) * N], ps[0:hs, :],
                AF.Identity, bias=b12_sb[0:hs, t:t + 1],
            )

        def emit_hy(t):
            hs = HSZ[t]
            psy = pp_pre.tile([128, ISH], f32, name="psy", tag="pre")
            for k in range(KD):
                nc.tensor.matmul(
                    psy[0:hs, :],
                    lhsT=w1y_sb[:, k * H + 128 * t: k * H + 128 * t + hs],
                    rhs=yx_sb[:, k * 2 * ISH:k * 2 * ISH + ISH],
                    start=(k == 0), stop=(k == KD - 1),
                )
            nc.vector.tensor_copy(hy_sb[0:hs, t * ISH:(t + 1) * ISH], psy[0:hs, :])

        for t in range(2):
            if t == 0:
                emit_hx(t)
                emit_hy(t)
            else:
                emit_hy(t)
                emit_hx(t)
            for g in range(STAG):
                for k4 in range(4):
                    emit_row_t(g, k4, t)

        # t2-pair staging: two rows' t2 relu args share one op.
        # Row pair (2p, 2p+1): partitions 0:44 carry row 2p's h=256..300
        # slice, partitions 44:88 carry row 2p+1's. The pair matvec uses a
        # [128, 33] lhsT whose col 0 / col 32 hold w2_t2 on the matching
        # partition ranges, so the two dots land on PSUM partitions
        # {base, base+32} — exactly the rows' accumulation slots.
        nc.gpsimd.memset(hxb2[:], 0.0)
        nc.gpsimd.memset(hy2[:], 0.0)
        ps2 = pp_pre.tile([128, N], f32, tag="pre")
        for k in range(KD):
            nc.tensor.matmul(
                ps2[0:108, :],
                lhsT=w1q_sb[:, k * 216:k * 216 + 108],
                rhs=xt_sb[:, k * N:(k + 1) * N],
                start=(k == 0), stop=(k == KD - 1),
            )
        nc.scalar.activation(
            hxb2[0:108, :], ps2[0:108, :],
            AF.Identity, bias=b12_sb[0:108, HT + 1:HT + 2],
        )
        psy2 = pp_pre.tile([128, ISH], f32, tag="pre")
        for k in range(KD):
            nc.tensor.matmul(
                psy2[0:108, :],
                lhsT=w1q_sb[:, k * 216 + 108:(k + 1) * 216],
                rhs=yx_sb[:, k * 2 * ISH:k * 2 * ISH + ISH],
                start=(k == 0), stop=(k == KD - 1),
            )
        nc.vector.tensor_copy(hy_sb[0:T2, 2 * ISH:3 * ISH], psy2[0:T2, :])
        psy2v = psy2[:, :].rearrange("p (a b) -> p a b", b=2)
        nc.vector.tensor_copy(hy2[0:T2, :], psy2v[0:T2, :, 0])
        nc.vector.tensor_copy(hy2[64:108, :], psy2v[64:108, :, 1])

        for g in range(STAG):
            for a in range(2):
                emit_pair(g, a)
            emit_exp(g)

        # ---- T0 partial from diagonal (only needs preamble) ----
        for t in range(HT):
            hs = HSZ[t]
            psd = pp_pre.tile([128, ISH], f32, tag="pre")
            for k in range(KD):
                nc.tensor.matmul(
                    psd[0:hs, :],
                    lhsT=w1x_sb[:, k * H + 128 * t: k * H + 128 * t + hs],
                    rhs=yx_sb[:, k * 2 * ISH + ISH:(k + 1) * 2 * ISH],
                    start=(k == 0), stop=(k == KD - 1),
                )
            nc.scalar.activation(
                hxd_sb[0:hs, t * ISH:(t + 1) * ISH], psd[0:hs, :],
                AF.Identity, bias=b12_sb[0:hs, t:t + 1],
            )
        dps = pp_misc.tile([128, ISH], f32, tag="dps")
        for t in range(HT):
            dsum = tpool.tile([128, ISH], f32, tag="dsum")
            nc.vector.tensor_add(
                dsum[:], hxd_sb[:, t * ISH:(t + 1) * ISH], hy_sb[:, t * ISH:(t + 1) * ISH]
            )
            dr = tpool.tile([128, ISH], bf16, tag="dr")
            nc.vector.tensor_scalar(dr[:], dsum[:], 0.0, None, ALU.max)
            nc.tensor.matmul(
                dps[0:1, :], lhsT=w2a_sb[:, t:t + 1], rhs=dr[:],
                start=(t == 0), stop=(t == HT - 1),
            )
        ed = tpool.tile([1, ISH], f32, tag="ed")
        nc.scalar.activation(ed[:], dps[0:1, :], AF.Exp, bias=b12_sb[0:1, HT:HT + 1])
        nc.sync.dma_start(out_ed[0:1, :], ed[0:1, :])

        # ---- main loop: remaining groups ----
        # bf16 relu tiles feed bf16 matvecs (1 PE cycle/row). DVE (2x 16-bit
        # mode) carries 8 relus per group; Act takes 2 plus the per-group Exp
        # that reduces the PSUM bank straight to row sums (accum_out), so z
        # never round-trips through SBUF.
        for g in range(STAG, NG):
            zps[g] = pp_z.tile([128, N], f32, name="zp", tag="zp")
            for k4 in range(4):
                for t in range(2):
                    emit_row_t(g, k4, t)
                if k4 == 1 and g > STAG:
                    emit_exp(g - 1)
                if k4 % 2 == 1:
                    emit_pair(g, k4 // 2)
            if g == NG - 1:
                emit_exp(g)

        # ---- tail: ship per-row exp sums; host does ln(N + sexp) ----
        nc.sync.dma_start(
            out_sx[0:4, :],
            sexp_all[:].rearrange("(a b) g -> a b g", b=32)[:, 0, :],
        )

        for p in (pp_misc, pp_z, pp_pre, pp_warm, tpool, rpool, cpool):
            p.release()

    nc.finalize()
    return nc


def _get_module():
    if "nc" not in _CACHE:
        _CACHE["nc"] = _build_module()
    return _CACHE["nc"]


def kernel(**inputs) -> np.ndarray:
    import ml_dtypes
    from concourse.bass_utils import run_bass_kernel_spmd

    bf16 = ml_dtypes.bfloat16
    x = np.ascontiguousarray(np.asarray(inputs["x_samples"], dtype=np.float32))
    y = np.ascontiguousarray(np.asarray(inputs["y_samples"], dtype=np.float32))
    W1 = np.asarray(inputs["W1"], dtype=np.float32)
    b1 = np.asarray(inputs["b1"], dtype=np.float32).reshape(H)
    W2 = np.asarray(inputs["W2"], dtype=np.float32)
    b2 = float(np.asarray(inputs["b2"], dtype=np.float32).reshape(1)[0])

    xT = np.ascontiguousarray(x.T.astype(bf16))                # [768, 512]
    w1xT = np.ascontiguousarray(W1[:, :XD].T.astype(bf16))     # [768, 300]
    w1yT = np.ascontiguousarray(W1[:, XD:].T.astype(bf16))     # [768, 300]

    b1p = np.zeros((128, HT), np.float32)
    w2p = np.zeros((128, HT), np.float32)
    w2 = W2.reshape(H)
    for t in range(HT):
        hs = HSZ[t]
        b1p[:hs, t] = b1[128 * t:128 * t + hs]
        w2p[:hs, t] = w2[128 * t:128 * t + hs]
    t2 = HSZ[2]
    w2qh = np.zeros((128, 64), np.float32)
    w2qh[:t2, 0] = w2[256:]
    w2qh[64:64 + t2, 32] = w2[256:]
    b1q = np.zeros((128, 1), np.float32)
    b1q[:t2, 0] = b1[256:]
    b1q[64:64 + t2, 0] = b1[256:]
    b12h = np.concatenate(
        [b1p, np.full((128, 1), b2, np.float32), b1q], axis=1)
    w2ah = np.concatenate([w2p, w2qh], axis=1)
    # duplicated t2 weights: cols 0:44 and 64:108 both carry the h=256..300
    # slice (x-half then y-half), matching the pair partition layout.
    w1qh = np.zeros((XD, 216), np.float32)
    w1qh[:, 0:t2] = W1[256:, :XD].T
    w1qh[:, 64:64 + t2] = W1[256:, :XD].T
    w1qh[:, 108:108 + t2] = W1[256:, XD:].T
    w1qh[:, 172:172 + t2] = W1[256:, XD:].T

    in_maps = []
    for c in range(NCORES):
        sl = slice(c * ISH, (c + 1) * ISH)
        in_maps.append({
            "xT": xT,
            "w1xT": w1xT,
            "w1yT": w1yT,
            "yx": np.ascontiguousarray(
                np.concatenate([y[sl].T, x[sl].T], axis=1).astype(bf16)),  # [768, 128]
            "b12": b12h,
            "w1q": np.ascontiguousarray(w1qh.astype(bf16)),
            "w2a": np.ascontiguousarray(w2ah.astype(bf16)),
        })

    nc = _get_module()
    res = run_bass_kernel_spmd(
        nc, in_maps, core_ids=list(range(NCORES)), trace=TRACE
    )
    global LAST_RESULTS
    LAST_RESULTS = res
    t0_sum = 0.0
    lse_sum = 0.0
    for r in res.results:
        ed = np.asarray(r["out_ed"], dtype=np.float64)
        sx = np.asarray(r["out_sx"], dtype=np.float64)
        t0_sum += float(np.log1p(ed).sum())
        lse_sum += float(np.log(float(N) + sx).sum())
    val = t0_sum / N - (lse_sum / N - math.log(N))
    return np.float32(val)


# revision 46
# speedup vs baseline: 1.0069x; 1.0069x over previous
"""InfoNCE lower-bound kernel for 8 Trainium2 NeuronCores.

Math (reference):
  hx = x @ W1x.T ; hy = y @ W1y.T            [N, H]
  z_ij = relu(hx[j] + hy[i] + b1) . w2       (logit WITHOUT b2)
  T1[i,j] = softplus(z_ij + b2)
  T0[i]   = T1[i,i]
  lse[i]  = log(sum_j exp(T1[i,j]))
  out     = mean(T0) - (mean(lse) - log N)

Key identity used on-device:  exp(softplus(v)) = 1 + e^v, so
  lse[i] = log(N + sum_j exp(z_ij + b2))
which avoids materializing softplus over the [N, N] grid.

Sharding: data-parallel over i (rows of the pair grid). Each of the 8
cores gets 64 rows (its slice of y), x and the MLP params replicated.
Per-core partial sums of T0 and lse are combined on the host.

The big [N/8, N, H] relu+matvec grid runs in bf16: the PE matvec
streams bf16 (1 cycle/row vs 4 for fp32) and the DVE relu ops hit the
2x 16-bit mode. PSUM accumulation stays fp32.

Per 4-row PSUM bank group: 8 relu+matvec ops cover h-tiles {128,128};
the 44-wide t2 tile packs two rows per op at partitions {0:44, 64:108}
(both ranges produced directly by matmuls against host-duplicated w1
slices — no staging DMAs), and a [128,33] lhsT whose cols 0/32 carry
w2_t2 on the matching ranges lands both dots on the rows' PSUM slots.
A per-group Exp with accum_out reduces each bank straight to row sums
— z never round-trips through SBUF — and the host finishes with
ln(N + sexp) / log1p(ed) on 8x(64+64) scalars. Engine budget per
group: DVE 8 relus (~2.1us), Act 2 relus + exp (~2.1us), PE 10
matvecs (~2.1us), all three ~saturated. The first 3 groups interleave
with the preamble; PE p-state warmup runs while inputs stream in.
Measured ~60-65us on 8 cores, rel err ~7e-4.
"""

import math

import numpy as np

N = 512
XD = 768
YD = 768
H = 300
NCORES = 8
ISH = N // NCORES  # 64 rows per core
KD = XD // 128     # 6 contraction tiles of 128
HT = 3             # h tiles: 128, 128, 44
HSZ = [128, 128, H - 256]

_CACHE = {}
TRACE = False
LAST_RESULTS = None


def _build_module():
    import concourse.bacc as bacc
    import concourse.mybir as mybir
    from concourse.tile import TileContext

    f32 = mybir.dt.float32
    bf16 = mybir.dt.bfloat16
    AF = mybir.ActivationFunctionType
    ALU = mybir.AluOpType
    AX = mybir.AxisListType

    nc = bacc.Bacc("TRN2", target_bir_lowering=False, debug=False)

    # Per-core inputs (SPMD: same shapes, different data for yT/xTd).
    xT = nc.dram_tensor("xT", [XD, N], bf16, kind="ExternalInput")       # x^T
    w1xT = nc.dram_tensor("w1xT", [XD, H], bf16, kind="ExternalInput")   # W1x^T
    w1yT = nc.dram_tensor("w1yT", [YD, H], bf16, kind="ExternalInput")   # W1y^T
    yx = nc.dram_tensor("yx", [YD, 2 * ISH], bf16, kind="ExternalInput")  # [y-slice^T | x-slice^T]
    b12 = nc.dram_tensor("b12", [128, HT + 2], f32, kind="ExternalInput")  # b1 packed | b2 | b1q
    w1q = nc.dram_tensor("w1q", [XD, 216], bf16, kind="ExternalInput")   # dup t2 weights [x|y]
    w2a = nc.dram_tensor("w2a", [128, HT + 64], bf16, kind="ExternalInput")  # w2 packed | t2-pair lhsT
    out_ed = nc.dram_tensor("out_ed", [1, ISH], f32, kind="ExternalOutput")      # exp(z_ii+b2)
    out_sx = nc.dram_tensor("out_sx", [4, ISH // 4], f32, kind="ExternalOutput")  # row exp sums

    with TileContext(nc) as tc:
        cpool = tc.alloc_tile_pool(name="consts", bufs=1)
        rpool = tc.alloc_tile_pool(name="work", bufs=24)
        tpool = tc.alloc_tile_pool(name="tail", bufs=1)
        pp_warm = tc.alloc_tile_pool(name="pp_warm", bufs=1, space="PSUM")
        pp_pre = tc.alloc_tile_pool(name="pp_pre", bufs=2, space="PSUM")
        pp_z = tc.alloc_tile_pool(name="pp_z", bufs=3, space="PSUM")
        pp_misc = tc.alloc_tile_pool(name="pp_misc", bufs=1, space="PSUM")

        # ---- load constants / inputs into SBUF ----
        # One fused DMA per DRAM tensor: each dma_start costs ~580ns of issue
        # on the SP queue, so 33 small loads would serialize ~19us.
        xt_sb = cpool.tile([128, KD * N], bf16, tag="xt")
        w1x_sb = cpool.tile([128, KD * H], bf16, tag="w1x")
        w1y_sb = cpool.tile([128, KD * H], bf16, tag="w1y")
        yx_sb = cpool.tile([128, KD * 2 * ISH], bf16, tag="yx")
        b12_sb = cpool.tile([128, HT + 2], f32, tag="b12")
        w1q_sb = cpool.tile([128, KD * 216], bf16, tag="w1q")
        w2a_sb = cpool.tile([128, HT + 64], bf16, tag="w2a")

        # Issue order tracks preamble consumption: xt+w1x[t0] unblock the
        # first hx matmuls; per-tile w1x/w1y slices arrive just in time for
        # each later tile. Queues drain in issue order, so one big DMA per
        # tensor would stall tile 0 on bytes only tile 2 needs.
        nc.sync.dma_start(
            xt_sb[:].rearrange("p (k c) -> p k c", k=KD),
            xT[:, :].rearrange("(k p) c -> p k c", p=128),
        )

        def load_w1_tile(dst_sb, src_dram, t):
            hs = HSZ[t]
            nc.sync.dma_start(
                dst_sb[:].rearrange("p (k c) -> p k c", k=KD)[:, :, 128 * t:128 * t + hs],
                src_dram[:, 128 * t:128 * t + hs].rearrange("(k p) c -> p k c", p=128),
            )

        load_w1_tile(w1x_sb, w1xT, 0)
        nc.sync.dma_start(b12_sb[:], b12[:])
        nc.sync.dma_start(w2a_sb[:], w2a[:])
        load_w1_tile(w1y_sb, w1yT, 0)
        nc.sync.dma_start(
            yx_sb[:].rearrange("p (k c) -> p k c", k=KD),
            yx[:, :].rearrange("(k p) c -> p k c", p=128),
        )
        load_w1_tile(w1x_sb, w1xT, 1)
        load_w1_tile(w1y_sb, w1yT, 1)
        nc.sync.dma_start(
            w1q_sb[:].rearrange("p (k c) -> p k c", k=KD),
            w1q[:, :].rearrange("(k p) c -> p k c", p=128),
        )
        load_w1_tile(w1x_sb, w1xT, 2)

        # ---- PE p-state warmup: dummy matmuls while input DMAs land ----
        # gpsimd memset has no upstream deps, so the PE starts within ~1us
        # of NEFF start and stays continuously busy (p-state ramps to 2.4GHz)
        # until the real preamble matmuls have data.
        warm_sb = cpool.tile([128, N], bf16, tag="warm")
        nc.gpsimd.memset(warm_sb[:], 0.0)
        wps = pp_warm.tile([128, N], f32, tag="wps")
        for _ in range(17):
            nc.tensor.matmul(
                wps[0:1, :], lhsT=warm_sb[:, 0:1], rhs=warm_sb[:],
                start=True, stop=True,
            )

        # ---- precompute hxT(+b1), hyT, hxdT on device ----
        hxb_sb = cpool.tile([128, 2 * N], bf16, tag="hxb")    # relu-arg x part (+b1)
        hy_sb = cpool.tile([128, HT * ISH], f32, tag="hy")    # y part (scalar operands)
        hxd_sb = cpool.tile([128, HT * ISH], f32, tag="hxd")  # diag x part (+b1)
        nc.vector.memset(hy_sb[:, 2 * ISH:3 * ISH], 0.0)
        nc.vector.memset(hxd_sb[:, 2 * ISH:3 * ISH], 0.0)

        NG = ISH // 4
        STAG = 3  # groups whose matvecs interleave with the preamble
        T2 = HSZ[2]
        sexp_all = cpool.tile([128, NG], f32, tag="sexp_all")
        ejunk = cpool.tile([128, N], bf16, tag="ejunk")
        hxb2 = cpool.tile([128, N], bf16, tag="hxb2")
        hy2 = cpool.tile([128, ISH // 2], f32, tag="hy2")
        zps = {}

        def emit_row_t(g, k4, t):
            i = 4 * g + k4
            r = rpool.tile([128, N], bf16, name="r", tag="r")
            col = hy_sb[:, t * ISH + i: t * ISH + i + 1]
            src = hxb_sb[:, t * N:(t + 1) * N]
            if t == 1 and k4 < 2:
                nc.scalar.activation(r[:], src, AF.Relu, bias=col)
            else:
                nc.vector.tensor_scalar(r[:], src, col, 0.0, ALU.add, ALU.max)
            nc.tensor.matmul(
                zps[g][32 * k4:32 * k4 + 1, :],
                lhsT=w2a_sb[:, t:t + 1], rhs=r[:],
                start=(t == 0), stop=False,
                tile_position=(0, 32 * k4),
                skip_group_check=True,
            )

        def emit_pair(g, a):
            pp = 2 * g + a
            r2 = rpool.tile([128, N], bf16, name="r2", tag="r2")
            nc.vector.tensor_scalar(
                r2[:], hxb2[:], hy2[:, pp:pp + 1], 0.0, ALU.add, ALU.max
            )
            nc.tensor.matmul(
                zps[g][64 * a:64 * a + 33, :],
                lhsT=w2a_sb[:, HT:HT + 33], rhs=r2[:],
                start=False, stop=True,
                tile_position=(0, 64 * a),
                skip_group_check=True,
            )

        def emit_exp(g):
            nc.scalar.activation(
                ejunk[:], zps[g][:], AF.Exp, bias=b12_sb[:, HT:HT + 1],
                accum_out=sexp_all[:, g:g + 1],
            )

        # ---- preamble interleaved with the first STAG groups ----
        # hx_t/hy_t clear tile t; the staged groups' tile-t matvecs then keep
        # the PE busy while the next tile's weights are still in flight.
        for g in range(STAG):
            zps[g] = pp_z.tile([128, N], f32, name="zp", tag="zp")
        def emit_hx(t):
            hs = HSZ[t]
            ps = pp_pre.tile([128, N], f32, name="ps", tag="pre")
            for k in range(KD):
                nc.tensor.matmul(
                    ps[0:hs, :],
                    lhsT=w1x_sb[:, k * H + 128 * t: k * H + 128 * t + hs],
                    rhs=xt_sb[:, k * N:(k + 1) * N],
                    start=(k == 0), stop=(k == KD - 1),
                )
            nc.scalar.activation(
                hxb_sb[0:hs, t * N:(t + 1) * N], ps[0:hs, :],
                AF.Identity, bias=b12_sb[0:hs, t:t + 1],
            )

        def emit_hy(t):
            hs = HSZ[t]
            psy = pp_pre.tile([128, ISH], f32, name="psy", tag="pre")
            for k in range(KD):
                nc.tensor.matmul(
                    psy[0:hs, :],
                    lhsT=w1y_sb[:, k * H + 128 * t: k * H + 128 * t + hs],
                    rhs=yx_sb[:, k * 2 * ISH:k * 2 * ISH + ISH],
                    start=(k == 0), stop=(k == KD - 1),
                )
            nc.vector.tensor_copy(hy_sb[0:hs, t * ISH:(t + 1) * ISH], psy[0:hs, :])

        for t in range(2):
            if t == 0:
                emit_hx(t)
                emit_hy(t)
                # fill the act->relu handoff latency before the first
                # staged matvec has an r tile ready
                for _ in range(3):
                    nc.tensor.matmul(
                        wps[0:1, :], lhsT=warm_sb[:, 0:1], rhs=warm_sb[:],
                        start=True, stop=True,
                    )
            else:
                emit_hy(t)
                emit_hx(t)
            for g in range(STAG):
                for k4 in range(4):
                    emit_row_t(g, k4, t)

        # t2-pair staging: two rows' t2 relu args share one op.
        # Row pair (2p, 2p+1): partitions 0:44 carry row 2p's h=256..300
        # slice, partitions 44:88 carry row 2p+1's. The pair matvec uses a
        # [128, 33] lhsT whose col 0 / col 32 hold w2_t2 on the matching
        # partition ranges, so the two dots land on PSUM partitions
        # {base, base+32} — exactly the rows' accumulation slots.
        nc.gpsimd.memset(hxb2[:], 0.0)
        nc.gpsimd.memset(hy2[:], 0.0)
        ps2 = pp_pre.tile([128, N], f32, tag="pre")
        for k in range(KD):
            nc.tensor.matmul(
                ps2[0:108, :],
                lhsT=w1q_sb[:, k * 216:k * 216 + 108],
                rhs=xt_sb[:, k * N:(k + 1) * N],
                start=(k == 0), stop=(k == KD - 1),
            )
        nc.scalar.activation(
            hxb2[0:108, :], ps2[0:108, :],
            AF.Identity, bias=b12_sb[0:108, HT + 1:HT + 2],
        )
        psy2 = pp_pre.tile([128, ISH], f32, tag="pre")
        for k in range(KD):
            nc.tensor.matmul(
                psy2[0:108, :],
                lhsT=w1q_sb[:, k * 216 + 108:(k + 1) * 216],
                rhs=yx_sb[:, k * 2 * ISH:k * 2 * ISH + ISH],
                start=(k == 0), stop=(k == KD - 1),
            )
        nc.vector.tensor_copy(hy_sb[0:T2, 2 * ISH:3 * ISH], psy2[0:T2, :])
        psy2v = psy2[:, :].rearrange("p (a b) -> p a b", b=2)
        nc.vector.tensor_copy(hy2[0:T2, :], psy2v[0:T2, :, 0])
        nc.vector.tensor_copy(hy2[64:108, :], psy2v[64:108, :, 1])

        for g in range(STAG):
            for a in range(2):
                emit_pair(g, a)
            emit_exp(g)

        # ---- T0 partial from diagonal (only needs preamble) ----
        for t in range(HT):
            hs = HSZ[t]
            psd = pp_pre.tile([128, ISH], f32, tag="pre")
            for k in range(KD):
                nc.tensor.matmul(
                    psd[0:hs, :],
                    lhsT=w1x_sb[:, k * H + 128 * t: k * H + 128 * t + hs],
                    rhs=yx_sb[:, k * 2 * ISH + ISH:(k + 1) * 2 * ISH],
                    start=(k == 0), stop=(k == KD - 1),
                )
            nc.scalar.activation(
                hxd_sb[0:hs, t * ISH:(t + 1) * ISH], psd[0:hs, :],
                AF.Identity, bias=b12_sb[0:hs, t:t + 1],
            )
        dps = pp_misc.tile([128, ISH], f32, tag="dps")
        for t in range(HT):
            dsum = tpool.tile([128, ISH], f32, tag="dsum")
            nc.vector.tensor_add(
                dsum[:], hxd_sb[:, t * ISH:(t + 1) * ISH], hy_sb[:, t * ISH:(t + 1) * ISH]
            )
            dr = tpool.tile([128, ISH], bf16, tag="dr")
            nc.vector.tensor_scalar(dr[:], dsum[:], 0.0, None, ALU.max)
            nc.tensor.matmul(
                dps[0:1, :], lhsT=w2a_sb[:, t:t + 1], rhs=dr[:],
                start=(t == 0), stop=(t == HT - 1),
            )
        ed = tpool.tile([1, ISH], f32, tag="ed")
        nc.scalar.activation(ed[:], dps[0:1, :], AF.Exp, bias=b12_sb[0:1, HT:HT + 1])
        nc.sync.dma_start(out_ed[0:1, :], ed[0:1, :])

        # ---- main loop: remaining groups ----
        # bf16 relu tiles feed bf16 matvecs (1 PE cycle/row). DVE (2x 16-bit
        # mode) carries 8 relus per group; Act takes 2 plus the per-group Exp
        # that reduces the PSUM bank straight to row sums (accum_out), so z
        # never round-trips through SBUF.
        for g in range(STAG, NG):
            zps[g] = pp_z.tile([128, N], f32, name="zp", tag="zp")
            for k4 in range(4):
                for t in range(2):
                    emit_row_t(g, k4, t)
                if k4 == 1 and g > STAG:
                    emit_exp(g - 1)
                if k4 % 2 == 1:
                    emit_pair(g, k4 // 2)
            if g == NG - 1:
                emit_exp(g)

        # ---- tail: ship per-row exp sums; host does ln(N + sexp) ----
        nc.sync.dma_start(
            out_sx[0:4, :],
            sexp_all[:].rearrange("(a b) g -> a b g", b=32)[:, 0, :],
        )

        for p in (pp_misc, pp_z, pp_pre, pp_warm, tpool, rpool, cpool):
            p.release()

    nc.finalize()
    return nc


def _get_module():
    if "nc" not in _CACHE:
        _CACHE["nc"] = _build_module()
    return _CACHE["nc"]


def kernel(**inputs) -> np.ndarray:
    import ml_dtypes
    from concourse.bass_utils import run_bass_kernel_spmd

    bf16 = ml_dtypes.bfloat16
    x = np.ascontiguousarray(np.asarray(inputs["x_samples"], dtype=np.float32))
    y = np.ascontiguousarray(np.asarray(inputs["y_samples"], dtype=np.float32))
    W1 = np.asarray(inputs["W1"], dtype=np.float32)
    b1 = np.asarray(inputs["b1"], dtype=np.float32).reshape(H)
    W2 = np.asarray(inputs["W2"], dtype=np.float32)
    b2 = float(np.asarray(inputs["b2"], dtype=np.float32).reshape(1)[0])

    xT = np.ascontiguousarray(x.T.astype(bf16))                # [768, 512]
    w1xT = np.ascontiguousarray(W1[:, :XD].T.astype(bf16))     # [768, 300]
    w1yT = np.ascontiguousarray(W1[:, XD:].T.astype(bf16))     # [768, 300]

    b1p = np.zeros((128, HT), np.float32)
    w2p = np.zeros((128, HT), np.float32)
    w2 = W2.reshape(H)
    for t in range(HT):
        hs = HSZ[t]
        b1p[:hs, t] = b1[128 * t:128 * t + hs]
        w2p[:hs, t] = w2[128 * t:128 * t + hs]
    t2 = HSZ[2]
    w2qh = np.zeros((128, 64), np.float32)
    w2qh[:t2, 0] = w2[256:]
    w2qh[64:64 + t2, 32] = w2[256:]
    b1q = np.zeros((128, 1), np.float32)
    b1q[:t2, 0] = b1[256:]
    b1q[64:64 + t2, 0] = b1[256:]
    b12h = np.concatenate(
        [b1p, np.full((128, 1), b2, np.float32), b1q], axis=1)
    w2ah = np.concatenate([w2p, w2qh], axis=1)
    # duplicated t2 weights: cols 0:44 and 64:108 both carry the h=256..300
    # slice (x-half then y-half), matching the pair partition layout.
    w1qh = np.zeros((XD, 216), np.float32)
    w1qh[:, 0:t2] = W1[256:, :XD].T
    w1qh[:, 64:64 + t2] = W1[256:, :XD].T
    w1qh[:, 108:108 + t2] = W1[256:, XD:].T
    w1qh[:, 172:172 + t2] = W1[256:, XD:].T

    in_maps = []
    for c in range(NCORES):
        sl = slice(c * ISH, (c + 1) * ISH)
        in_maps.append({
            "xT": xT,
            "w1xT": w1xT,
            "w1yT": w1yT,
            "yx": np.ascontiguousarray(
                np.concatenate([y[sl].T, x[sl].T], axis=1).astype(bf16)),  # [768, 128]
            "b12": b12h,
            "w1q": np.ascontiguousarray(w1qh.astype(bf16)),
            "w2a": np.ascontiguousarray(w2ah.astype(bf16)),
        })

    nc = _get_module()
    res = run_bass_kernel_spmd(
        nc, in_maps, core_ids=list(range(NCORES)), trace=TRACE
    )
    global LAST_RESULTS
    LAST_RESULTS = res
    t0_sum = 0.0
    lse_sum = 0.0
    for r in res.results:
        ed = np.asarray(r["out_ed"], dtype=np.float64)
        sx = np.asarray(r["out_sx"], dtype=np.float64)
        t0_sum += float(np.log1p(ed).sum())
        lse_sum += float(np.log(float(N) + sx).sum())
    val = t0_sum / N - (lse_sum / N - math.log(N))
    return np.float32(val)


# revision 47
# speedup vs baseline: 1.0071x; 1.0002x over previous
"""InfoNCE lower-bound kernel for 8 Trainium2 NeuronCores.

Math (reference):
  hx = x @ W1x.T ; hy = y @ W1y.T            [N, H]
  z_ij = relu(hx[j] + hy[i] + b1) . w2       (logit WITHOUT b2)
  T1[i,j] = softplus(z_ij + b2)
  T0[i]   = T1[i,i]
  lse[i]  = log(sum_j exp(T1[i,j]))
  out     = mean(T0) - (mean(lse) - log N)

Key identity used on-device:  exp(softplus(v)) = 1 + e^v, so
  lse[i] = log(N + sum_j exp(z_ij + b2))
which avoids materializing softplus over the [N, N] grid.

Sharding: data-parallel over i (rows of the pair grid). Each of the 8
cores gets 64 rows (its slice of y), x and the MLP params replicated.
Per-core partial sums of T0 and lse are combined on the host.

The big [N/8, N, H] relu+matvec grid runs in bf16: the PE matvec
streams bf16 (1 cycle/row vs 4 for fp32) and the DVE relu ops hit the
2x 16-bit mode. PSUM accumulation stays fp32.

Per 4-row PSUM bank group: 8 relu+matvec ops cover h-tiles {128,128};
the 44-wide t2 tile packs two rows per op at partitions {0:44, 64:108}
(both ranges produced directly by matmuls against host-duplicated w1
slices — no staging DMAs), and a [128,33] lhsT whose cols 0/32 carry
w2_t2 on the matching ranges lands both dots on the rows' PSUM slots.
A per-group Exp with accum_out reduces each bank straight to row sums
— z never round-trips through SBUF — and the host finishes with
ln(N + sexp) / log1p(ed) on 8x(64+64) scalars. Engine budget per
group: DVE 8 relus (~2.1us), Act 2 relus + exp (~2.1us), PE 10
matvecs (~2.1us), all three ~saturated. The first 3 groups interleave
with the preamble; PE p-state warmup runs while inputs stream in.
Measured ~60-65us on 8 cores, rel err ~7e-4.
"""

import math

import numpy as np

N = 512
XD = 768
YD = 768
H = 300
NCORES = 8
ISH = N // NCORES  # 64 rows per core
KD = XD // 128     # 6 contraction tiles of 128
HT = 3             # h tiles: 128, 128, 44
HSZ = [128, 128, H - 256]

_CACHE = {}
TRACE = False
LAST_RESULTS = None


def _build_module():
    import concourse.bacc as bacc
    import concourse.mybir as mybir
    from concourse.tile import TileContext

    f32 = mybir.dt.float32
    bf16 = mybir.dt.bfloat16
    AF = mybir.ActivationFunctionType
    ALU = mybir.AluOpType
    AX = mybir.AxisListType

    nc = bacc.Bacc("TRN2", target_bir_lowering=False, debug=False)

    # Per-core inputs (SPMD: same shapes, different data for yT/xTd).
    xT = nc.dram_tensor("xT", [XD, N], bf16, kind="ExternalInput")       # x^T
    w1xT = nc.dram_tensor("w1xT", [XD, H], bf16, kind="ExternalInput")   # W1x^T
    w1yT = nc.dram_tensor("w1yT", [YD, H], bf16, kind="ExternalInput")   # W1y^T
    yx = nc.dram_tensor("yx", [YD, 2 * ISH], bf16, kind="ExternalInput")  # [y-slice^T | x-slice^T]
    b12 = nc.dram_tensor("b12", [128, HT + 2], f32, kind="ExternalInput")  # b1 packed | b2 | b1q
    w1q = nc.dram_tensor("w1q", [XD, 216], bf16, kind="ExternalInput")   # dup t2 weights [x|y]
    w2a = nc.dram_tensor("w2a", [128, HT + 64], bf16, kind="ExternalInput")  # w2 packed | t2-pair lhsT
    out_ed = nc.dram_tensor("out_ed", [1, ISH], f32, kind="ExternalOutput")      # exp(z_ii+b2)
    out_sx = nc.dram_tensor("out_sx", [4, ISH // 4], f32, kind="ExternalOutput")  # row exp sums

    with TileContext(nc) as tc:
        cpool = tc.alloc_tile_pool(name="consts", bufs=1)
        rpool = tc.alloc_tile_pool(name="work", bufs=24)
        tpool = tc.alloc_tile_pool(name="tail", bufs=1)
        pp_warm = tc.alloc_tile_pool(name="pp_warm", bufs=1, space="PSUM")
        pp_pre = tc.alloc_tile_pool(name="pp_pre", bufs=2, space="PSUM")
        pp_z = tc.alloc_tile_pool(name="pp_z", bufs=4, space="PSUM")

        # ---- load constants / inputs into SBUF ----
        # One fused DMA per DRAM tensor: each dma_start costs ~580ns of issue
        # on the SP queue, so 33 small loads would serialize ~19us.
        xt_sb = cpool.tile([128, KD * N], bf16, tag="xt")
        w1x_sb = cpool.tile([128, KD * H], bf16, tag="w1x")
        w1y_sb = cpool.tile([128, KD * H], bf16, tag="w1y")
        yx_sb = cpool.tile([128, KD * 2 * ISH], bf16, tag="yx")
        b12_sb = cpool.tile([128, HT + 2], f32, tag="b12")
        w1q_sb = cpool.tile([128, KD * 216], bf16, tag="w1q")
        w2a_sb = cpool.tile([128, HT + 64], bf16, tag="w2a")

        # Issue order tracks preamble consumption: xt+w1x[t0] unblock the
        # first hx matmuls; per-tile w1x/w1y slices arrive just in time for
        # each later tile. Queues drain in issue order, so one big DMA per
        # tensor would stall tile 0 on bytes only tile 2 needs.
        nc.sync.dma_start(
            xt_sb[:].rearrange("p (k c) -> p k c", k=KD),
            xT[:, :].rearrange("(k p) c -> p k c", p=128),
        )

        def load_w1_tile(dst_sb, src_dram, t):
            hs = HSZ[t]
            nc.sync.dma_start(
                dst_sb[:].rearrange("p (k c) -> p k c", k=KD)[:, :, 128 * t:128 * t + hs],
                src_dram[:, 128 * t:128 * t + hs].rearrange("(k p) c -> p k c", p=128),
            )

        load_w1_tile(w1x_sb, w1xT, 0)
        nc.sync.dma_start(b12_sb[:], b12[:])
        nc.sync.dma_start(w2a_sb[:], w2a[:])
        load_w1_tile(w1y_sb, w1yT, 0)
        nc.sync.dma_start(
            yx_sb[:].rearrange("p (k c) -> p k c", k=KD),
            yx[:, :].rearrange("(k p) c -> p k c", p=128),
        )
        load_w1_tile(w1x_sb, w1xT, 1)
        load_w1_tile(w1y_sb, w1yT, 1)
        nc.sync.dma_start(
            w1q_sb[:].rearrange("p (k c) -> p k c", k=KD),
            w1q[:, :].rearrange("(k p) c -> p k c", p=128),
        )
        load_w1_tile(w1x_sb, w1xT, 2)

        # ---- PE p-state warmup: dummy matmuls while input DMAs land ----
        # gpsimd memset has no upstream deps, so the PE starts within ~1us
        # of NEFF start and stays continuously busy (p-state ramps to 2.4GHz)
        # until the real preamble matmuls have data.
        warm_sb = cpool.tile([128, N], bf16, tag="warm")
        nc.gpsimd.memset(warm_sb[:], 0.0)
        wps = pp_warm.tile([128, N], f32, tag="wps")
        for _ in range(17):
            nc.tensor.matmul(
                wps[0:1, :], lhsT=warm_sb[:, 0:1], rhs=warm_sb[:],
                start=True, stop=True,
            )

        # ---- precompute hxT(+b1), hyT, hxdT on device ----
        hxb_sb = cpool.tile([128, 2 * N], bf16, tag="hxb")    # relu-arg x part (+b1)
        hy_sb = cpool.tile([128, HT * ISH], f32, tag="hy")    # y part (scalar operands)
        hxd_sb = cpool.tile([128, HT * ISH], f32, tag="hxd")  # diag x part (+b1)
        nc.vector.memset(hy_sb[:, 2 * ISH:3 * ISH], 0.0)
        nc.vector.memset(hxd_sb[:, 2 * ISH:3 * ISH], 0.0)

        NG = ISH // 4
        STAG = 3  # groups whose matvecs interleave with the preamble
        T2 = HSZ[2]
        sexp_all = cpool.tile([128, NG], f32, tag="sexp_all")
        ejunk = cpool.tile([128, N], bf16, tag="ejunk")
        hxb2 = cpool.tile([128, N], bf16, tag="hxb2")
        hy2 = cpool.tile([128, ISH // 2], f32, tag="hy2")
        zps = {}

        def emit_row_t(g, k4, t):
            i = 4 * g + k4
            r = rpool.tile([128, N], bf16, name="r", tag="r")
            col = hy_sb[:, t * ISH + i: t * ISH + i + 1]
            src = hxb_sb[:, t * N:(t + 1) * N]
            if t == 1 and k4 < 2:
                nc.scalar.activation(r[:], src, AF.Relu, bias=col)
            else:
                nc.vector.tensor_scalar(r[:], src, col, 0.0, ALU.add, ALU.max)
            nc.tensor.matmul(
                zps[g][32 * k4:32 * k4 + 1, :],
                lhsT=w2a_sb[:, t:t + 1], rhs=r[:],
                start=(t == 0), stop=False,
                tile_position=(0, 32 * k4),
                skip_group_check=True,
            )

        def emit_pair(g, a):
            pp = 2 * g + a
            r2 = rpool.tile([128, N], bf16, name="r2", tag="r2")
            nc.vector.tensor_scalar(
                r2[:], hxb2[:], hy2[:, pp:pp + 1], 0.0, ALU.add, ALU.max
            )
            nc.tensor.matmul(
                zps[g][64 * a:64 * a + 33, :],
                lhsT=w2a_sb[:, HT:HT + 33], rhs=r2[:],
                start=False, stop=True,
                tile_position=(0, 64 * a),
                skip_group_check=True,
            )

        def emit_exp(g):
            nc.scalar.activation(
                ejunk[:], zps[g][:], AF.Exp, bias=b12_sb[:, HT:HT + 1],
                accum_out=sexp_all[:, g:g + 1],
            )

        # ---- preamble interleaved with the first STAG groups ----
        # hx_t/hy_t clear tile t; the staged groups' tile-t matvecs then keep
        # the PE busy while the next tile's weights are still in flight.
        for g in range(STAG):
            zps[g] = pp_z.tile([128, N], f32, name="zp", tag="zp")
        def emit_hx(t):
            hs = HSZ[t]
            ps = pp_pre.tile([128, N], f32, name="ps", tag="pre")
            for k in range(KD):
                nc.tensor.matmul(
                    ps[0:hs, :],
                    lhsT=w1x_sb[:, k * H + 128 * t: k * H + 128 * t + hs],
                    rhs=xt_sb[:, k * N:(k + 1) * N],
                    start=(k == 0), stop=(k == KD - 1),
                )
            nc.scalar.activation(
                hxb_sb[0:hs, t * N:(t + 1) * N], ps[0:hs, :],
                AF.Identity, bias=b12_sb[0:hs, t:t + 1],
            )

        def emit_hy(t):
            hs = HSZ[t]
            psy = pp_pre.tile([128, ISH], f32, name="psy", tag="pre")
            for k in range(KD):
                nc.tensor.matmul(
                    psy[0:hs, :],
                    lhsT=w1y_sb[:, k * H + 128 * t: k * H + 128 * t + hs],
                    rhs=yx_sb[:, k * 2 * ISH:k * 2 * ISH + ISH],
                    start=(k == 0), stop=(k == KD - 1),
                )
            nc.vector.tensor_copy(hy_sb[0:hs, t * ISH:(t + 1) * ISH], psy[0:hs, :])

        for t in range(2):
            if t == 0:
                emit_hx(t)
                emit_hy(t)
                # fill the act->relu handoff latency before the first
                # staged matvec has an r tile ready
                for _ in range(3):
                    nc.tensor.matmul(
                        wps[0:1, :], lhsT=warm_sb[:, 0:1], rhs=warm_sb[:],
                        start=True, stop=True,
                    )
            else:
                emit_hy(t)
                emit_hx(t)
            for g in range(STAG):
                for k4 in range(4):
                    emit_row_t(g, k4, t)

        # t2-pair staging: two rows' t2 relu args share one op.
        # Row pair (2p, 2p+1): partitions 0:44 carry row 2p's h=256..300
        # slice, partitions 44:88 carry row 2p+1's. The pair matvec uses a
        # [128, 33] lhsT whose col 0 / col 32 hold w2_t2 on the matching
        # partition ranges, so the two dots land on PSUM partitions
        # {base, base+32} — exactly the rows' accumulation slots.
        nc.gpsimd.memset(hxb2[:], 0.0)
        nc.gpsimd.memset(hy2[:], 0.0)
        ps2 = pp_pre.tile([128, N], f32, tag="pre")
        for k in range(KD):
            nc.tensor.matmul(
                ps2[0:108, :],
                lhsT=w1q_sb[:, k * 216:k * 216 + 108],
                rhs=xt_sb[:, k * N:(k + 1) * N],
                start=(k == 0), stop=(k == KD - 1),
            )
        nc.scalar.activation(
            hxb2[0:108, :], ps2[0:108, :],
            AF.Identity, bias=b12_sb[0:108, HT + 1:HT + 2],
        )
        psy2 = pp_pre.tile([128, ISH], f32, tag="pre")
        for k in range(KD):
            nc.tensor.matmul(
                psy2[0:108, :],
                lhsT=w1q_sb[:, k * 216 + 108:(k + 1) * 216],
                rhs=yx_sb[:, k * 2 * ISH:k * 2 * ISH + ISH],
                start=(k == 0), stop=(k == KD - 1),
            )
        nc.vector.tensor_copy(hy_sb[0:T2, 2 * ISH:3 * ISH], psy2[0:T2, :])
        psy2v = psy2[:, :].rearrange("p (a b) -> p a b", b=2)
        nc.vector.tensor_copy(hy2[0:T2, :], psy2v[0:T2, :, 0])
        nc.vector.tensor_copy(hy2[64:108, :], psy2v[64:108, :, 1])

        for g in range(STAG):
            for a in range(2):
                emit_pair(g, a)
            emit_exp(g)

        # ---- T0 partial from diagonal (only needs preamble) ----
        for t in range(HT):
            hs = HSZ[t]
            psd = pp_pre.tile([128, ISH], f32, tag="pre")
            for k in range(KD):
                nc.tensor.matmul(
                    psd[0:hs, :],
                    lhsT=w1x_sb[:, k * H + 128 * t: k * H + 128 * t + hs],
                    rhs=yx_sb[:, k * 2 * ISH + ISH:(k + 1) * 2 * ISH],
                    start=(k == 0), stop=(k == KD - 1),
                )
            nc.scalar.activation(
                hxd_sb[0:hs, t * ISH:(t + 1) * ISH], psd[0:hs, :],
                AF.Identity, bias=b12_sb[0:hs, t:t + 1],
            )
        dps = pp_warm.tile([128, ISH], f32, tag="wps")
        for t in range(HT):
            dsum = tpool.tile([128, ISH], f32, tag="dsum")
            nc.vector.tensor_add(
                dsum[:], hxd_sb[:, t * ISH:(t + 1) * ISH], hy_sb[:, t * ISH:(t + 1) * ISH]
            )
            dr = tpool.tile([128, ISH], bf16, tag="dr")
            nc.vector.tensor_scalar(dr[:], dsum[:], 0.0, None, ALU.max)
            nc.tensor.matmul(
                dps[0:1, :], lhsT=w2a_sb[:, t:t + 1], rhs=dr[:],
                start=(t == 0), stop=(t == HT - 1),
            )
        ed = tpool.tile([1, ISH], f32, tag="ed")
        nc.scalar.activation(ed[:], dps[0:1, :], AF.Exp, bias=b12_sb[0:1, HT:HT + 1])
        nc.sync.dma_start(out_ed[0:1, :], ed[0:1, :])

        # ---- main loop: remaining groups ----
        # bf16 relu tiles feed bf16 matvecs (1 PE cycle/row). DVE (2x 16-bit
        # mode) carries 8 relus per group; Act takes 2 plus the per-group Exp
        # that reduces the PSUM bank straight to row sums (accum_out), so z
        # never round-trips through SBUF.
        for g in range(STAG, NG):
            zps[g] = pp_z.tile([128, N], f32, name="zp", tag="zp")
            for k4 in range(4):
                for t in range(2):
                    emit_row_t(g, k4, t)
                if k4 == 1 and g > STAG:
                    emit_exp(g - 1)
                if k4 % 2 == 1:
                    emit_pair(g, k4 // 2)
            if g == NG - 1:
                emit_exp(g)

        # ---- tail: ship per-row exp sums; host does ln(N + sexp) ----
        nc.sync.dma_start(
            out_sx[0:4, :],
            sexp_all[:].rearrange("(a b) g -> a b g", b=32)[:, 0, :],
        )

        for p in (pp_z, pp_pre, pp_warm, tpool, rpool, cpool):
            p.release()

    nc.finalize()
    return nc


def _get_module():
    if "nc" not in _CACHE:
        _CACHE["nc"] = _build_module()
    return _CACHE["nc"]


def kernel(**inputs) -> np.ndarray:
    import ml_dtypes
    from concourse.bass_utils import run_bass_kernel_spmd

    bf16 = ml_dtypes.bfloat16
    x = np.ascontiguousarray(np.asarray(inputs["x_samples"], dtype=np.float32))
    y = np.ascontiguousarray(np.asarray(inputs["y_samples"], dtype=np.float32))
    W1 = np.asarray(inputs["W1"], dtype=np.float32)
    b1 = np.asarray(inputs["b1"], dtype=np.float32).reshape(H)
    W2 = np.asarray(inputs["W2"], dtype=np.float32)
    b2 = float(np.asarray(inputs["b2"], dtype=np.float32).reshape(1)[0])

    xT = np.ascontiguousarray(x.T.astype(bf16))                # [768, 512]
    w1xT = np.ascontiguousarray(W1[:, :XD].T.astype(bf16))     # [768, 300]
    w1yT = np.ascontiguousarray(W1[:, XD:].T.astype(bf16))     # [768, 300]

    b1p = np.zeros((128, HT), np.float32)
    w2p = np.zeros((128, HT), np.float32)
    w2 = W2.reshape(H)
    for t in range(HT):
        hs = HSZ[t]
        b1p[:hs, t] = b1[128 * t:128 * t + hs]
        w2p[:hs, t] = w2[128 * t:128 * t + hs]
    t2 = HSZ[2]
    w2qh = np.zeros((128, 64), np.float32)
    w2qh[:t2, 0] = w2[256:]
    w2qh[64:64 + t2, 32] = w2[256:]
    b1q = np.zeros((128, 1), np.float32)
    b1q[:t2, 0] = b1[256:]
    b1q[64:64 + t2, 0] = b1[256:]
    b12h = np.concatenate(
        [b1p, np.full((128, 1), b2, np.float32), b1q], axis=1)
    w2ah = np.concatenate([w2p, w2qh], axis=1)
    # duplicated t2 weights: cols 0:44 and 64:108 both carry the h=256..300
    # slice (x-half then y-half), matching the pair partition layout.
    w1qh = np.zeros((XD, 216), np.float32)
    w1qh[:, 0:t2] = W1[256:, :XD].T
    w1qh[:, 64:64 + t2] = W1[256:, :XD].T
    w1qh[:, 108:108 + t2] = W1[256:, XD:].T
    w1qh[:, 172:172 + t2] = W1[256:, XD:].T

    in_maps = []
    for c in range(NCORES):
        sl = slice(c * ISH, (c + 1) * ISH)
        in_maps.append({
            "xT": xT,
            "w1xT": w1xT,
            "w1yT": w1yT,
            "yx": np.ascontiguousarray(
                np.concatenate([y[sl].T, x[sl].T], axis=1).astype(bf16)),  # [768, 128]
            "b12": b12h,
            "w1q": np.ascontiguousarray(w1qh.astype(bf16)),
            "w2a": np.ascontiguousarray(w2ah.astype(bf16)),
        })

    nc = _get_module()
    res = run_bass_kernel_spmd(
        nc, in_maps, core_ids=list(range(NCORES)), trace=TRACE
    )
    global LAST_RESULTS
    LAST_RESULTS = res
    t0_sum = 0.0
    lse_sum = 0.0
    for r in res.results:
        ed = np.asarray(r["out_ed"], dtype=np.float64)
        sx = np.asarray(r["out_sx"], dtype=np.float64)
        t0_sum += float(np.log1p(ed).sum())
        lse_sum += float(np.log(float(N) + sx).sum())
    val = t0_sum / N - (lse_sum / N - math.log(N))
    return np.float32(val)
